# revision 3
# baseline (speedup 1.0000x reference)
"""Bidirectional attention (RoPE-variant) Trainium2 kernel.

Reference computation (B=4, T=2048, C=2048, H=16, D=128):
    q = (x @ wq.T) -> rotary; k = (x @ wk.T) -> rotary; v = x @ wv.T
    y = softmax(q k^T / sqrt(D)) v ; out = y @ wo.T

Sharding over 8 NeuronCores: core c -> (batch b = c//2, head-group g = c%2).
Each core computes q/k/v projections for its batch restricted to its 8 heads,
full attention for those heads, and a partial o-projection (contracting its
1024 hidden columns).  The host sums the two partial outputs per batch — no
device collectives, and every core does exactly 1/8 of the matmul FLOPs
(3072 matmuls of 128x128x512, the bf16/fp16 roofline).

Everything is fp16 (NOT bf16): the PE runs fp16 at the same rate as bf16
(both upconvert to e10m11 internally, where fp16 is exact), and fp16's 10
mantissa bits put the whole pipeline at ~1.4e-3 max-rel error without the
split-K hi/lo trick a bf16 kernel needs for the exp-amplified scores path
(bf16 no-split measures 1.5e-2 — too close to the 2e-2 gate).  The only
range hazard is exp(score): scores reach +-20.7 for this data, so the exp
activation gets bias=-11 (softmax is shift-invariant; the shift cancels in
the normalizer) keeping u = exp(s-11) <= ~1.6e4 < fp16 max 65504.

Schedule: V is computed first and spilled to DRAM per head (contiguous
reload); Q^T/K^T stay resident in SBUF (no spill round-trip).  Attention for
head h-1 is emitted between the Q/K blocks of head h so its exp() (ScalarE)
and tree-sum/gpsimd/reciprocal work hides under projection matmuls; within
attention, scores+exp for chunk qc+1 are emitted before AV(qc) so the ACT
drain never stalls the PE.  The o-projection is interleaved into the last
head's attention, with the head-7 yt contribution accumulated last in each
chain so the softmax tail latency hides under the first 28 matmuls.  Input
loads are issued in first-use order (wv chunk 0, then x tile-major) so the
first matmul starts ~10us in instead of ~70us.
"""

import sys

if "/opt/trn_rl_repo" not in sys.path:
    sys.path.insert(0, "/opt/trn_rl_repo")

import numpy as np

B, T, C = 4, 2048, 2048
H_TOT = 16
D = 128
HG = 8            # heads per core
JG = HG * D       # 1024 hidden columns per head-group
N_CORES = 8
CT = C // 128     # 16 c-tiles (contraction over channels)
TT = T // 128     # 16 t-tiles
QCH = T // 512    # 4 query chunks of 512
KT = T // 128     # 16 key tiles of 128
SCALE = 1.0 / float(np.sqrt(D))
SHIFT = -11.0     # exp(s*SCALE + SHIFT); cancels in the softmax normalizer

F16 = np.float16

_CACHE = {}


def _build_bass():
    import concourse.tile as tile
    import concourse.bass_isa as bass_isa
    from concourse import bacc, mybir
    from concourse.bass import ts
    from contextlib import ExitStack

    f16 = mybir.dt.float16
    f32 = mybir.dt.float32

    nc = bacc.Bacc("TRN2", target_bir_lowering=False, debug=False)

    # register the exp-shift constant (same pattern as Bass.__init__ consts)
    shift_t = nc.alloc_sbuf_tensor(f"const-float32-{SHIFT}", [128, 1], f32)
    nc.gpsimd.memset(shift_t.ap(), SHIFT)
    nc.const_aps.aps[(f32, SHIFT)] = shift_t.ap()
    nc.all_engine_barrier()

    # x_pack[ci, tm, co, tq] = x[b, tm*128+tq, co*128+ci] — per-tm contiguous
    x_pack = nc.dram_tensor("x_pack", [128, TT, CT, 128], f16, kind="ExternalInput")
    wq_pack = nc.dram_tensor("wq_pack", [HG, 128, CT, 128], f16, kind="ExternalInput")
    wk_pack = nc.dram_tensor("wk_pack", [HG, 128, CT, 128], f16, kind="ExternalInput")
    # wv_pack[ci, dch, co, dq] = wv_g[dch*512+dq, co*128+ci] — per-dch contiguous
    wv_pack = nc.dram_tensor("wv_pack", [128, 2, CT, 512], f16, kind="ExternalInput")
    wo_pack = nc.dram_tensor("wo_pack", [128, HG, C], f16, kind="ExternalInput")
    # cs_pack rows 0:64 = cos^T, rows 64:128 = sin^T
    cs_pack = nc.dram_tensor("cs_pack", [128, T], f16, kind="ExternalInput")
    out = nc.dram_tensor("out", [T, C], f32, kind="ExternalOutput")

    with tile.TileContext(nc) as tc, ExitStack() as ctx:
        # Pools opened in lifetime order: persistent + attention scratch first
        # (bottom of the SBUF stack), then phase-1 pools on top, so attention
        # tiles never alias phase-1 space.
        persist = ctx.enter_context(tc.tile_pool(name="persist", bufs=1))
        p2k = ctx.enter_context(tc.tile_pool(name="p2k", bufs=2))
        p2q = ctx.enter_context(tc.tile_pool(name="p2q", bufs=2))
        p2u = ctx.enter_context(tc.tile_pool(name="p2u", bufs=2))
        p2sm = ctx.enter_context(tc.tile_pool(name="p2sm", bufs=1))
        p2v = ctx.enter_context(tc.tile_pool(name="p2v", bufs=1))
        dram = ctx.enter_context(tc.tile_pool(name="dram", bufs=1, space="DRAM"))
        ps_sc = ctx.enter_context(tc.tile_pool(name="ps_sc", bufs=2, space="PSUM"))
        ps_gen = ctx.enter_context(tc.tile_pool(name="ps_gen", bufs=4, space="PSUM"))

        yt_sb = persist.tile([128, HG, T], f16)      # y^T, (d, h, t)

        v_dram = [
            dram.tile([128, TT, 128], f16, tag=f"v{h}", name=f"v_dram{h}")
            for h in range(HG)
        ]

        def oproj_group(g):
            # o-proj tile-group g needs q-chunk g of ALL heads; emitted inside
            # attention(7) right after ymul(g).  ji ascending puts the head-7
            # dependency at matmul 28/32 of the first chain, hiding the
            # softmax tail latency.
            for tm in range(4 * g, 4 * g + 4):
                pss = [
                    ps_gen.tile([128, 512], f32, tag="ps", name=f"pso{tm}_{c}")
                    for c in range(C // 512)
                ]
                for ji in range(HG):
                    for cch in range(C // 512):
                        nc.tensor.matmul(
                            pss[cch][:],
                            lhsT=yt_sb[:, ji, ts(tm, 128)],
                            rhs=wo_sb[:, ji, ts(cch, 512)],
                            start=(ji == 0),
                            stop=(ji == HG - 1),
                        )
                for cch in range(C // 512):
                    stg = p3stg.tile(
                        [128, 512], f32, tag="ostg", bufs=4,
                        name=f"ostg{tm}_{cch}"
                    )
                    nc.vector.tensor_copy(out=stg[:], in_=pss[cch][:])
                    nc.sync.dma_start(
                        out=out.ap()[ts(tm, 128), ts(cch, 512)], in_=stg[:]
                    )

        def attention(h, qt, kt, tail=False):
            v_h = p2v.tile([128, TT, 128], f16, tag="vh", name=f"vh{h}")
            nc.sync.dma_start(out=v_h[:], in_=v_dram[h][:])

            def sc_exp(qc):
                u = p2u.tile(
                    [128, KT // 2, 2, 512], f16, tag="u", name=f"u{h}_{qc}"
                )
                for kg in range(KT // 2):
                    ps = ps_sc.tile(
                        [128, 2, 512], f32, tag="ps", name=f"sc{h}_{qc}_{kg}"
                    )
                    for kk in range(2):
                        nc.tensor.matmul(
                            ps[:, kk, :],
                            lhsT=kt[:, ts(2 * kg + kk, 128)],
                            rhs=qt[:, ts(qc, 512)],
                            start=True,
                            stop=True,
                        )
                    nc.scalar.activation(
                        out=u[:, kg, :, :],
                        in_=ps[:],
                        func=mybir.ActivationFunctionType.Exp,
                        scale=SCALE,
                        bias=SHIFT,
                    )
                return u

            def softmax_tail(qc, u):
                # denominator: tree-sum over the 16 k-tiles, then partitions
                s8 = p2sm.tile([128, 8, 512], f16, tag="s8", name=f"s8_{h}{qc}")
                nc.vector.tensor_add(s8[:], u[:, :, 0, :], u[:, :, 1, :])
                s8v = s8[:].rearrange("p (x y) q -> p x y q", x=4)
                s4 = p2sm.tile([128, 4, 512], f16, tag="s4", name=f"s4_{h}{qc}")
                nc.vector.tensor_add(s4[:], s8v[:, :, 0, :], s8v[:, :, 1, :])
                s4v = s4[:].rearrange("p (x y) q -> p x y q", x=2)
                s2r = p2sm.tile([128, 3, 512], f16, tag="s2r", name=f"s2r_{h}{qc}")
                nc.vector.tensor_add(
                    s2r[:, 0:2, :], s4v[:, :, 0, :], s4v[:, :, 1, :]
                )
                nc.vector.tensor_add(s2r[:, 2, :], s2r[:, 0, :], s2r[:, 1, :])
                rsum = p2sm.tile([128, 512], f32, tag="s8", name=f"rs_{h}{qc}")
                nc.gpsimd.partition_all_reduce(
                    rsum[:], s2r[:, 2, :], channels=128,
                    reduce_op=bass_isa.ReduceOp.add
                )
                rrec = p2sm.tile([128, 512], f32, tag="s4", name=f"rr_{h}{qc}")
                nc.vector.reciprocal_approx_fast(out=rrec[:], in_=rsum[:])
                return rrec

            def av(qc, u, rrec):
                psy = ps_gen.tile([128, 512], f32, tag="ps", name=f"psy{h}_{qc}")
                for kt_i in range(KT):
                    nc.tensor.matmul(
                        psy[:],
                        lhsT=v_h[:, kt_i, :],
                        rhs=u[:, kt_i // 2, kt_i % 2, :],
                        start=(kt_i == 0),
                        stop=(kt_i == KT - 1),
                    )
                nc.vector.tensor_mul(
                    out=yt_sb[:, h, ts(qc, 512)], in0=psy[:], in1=rrec[:]
                )

            u_cur = sc_exp(0)
            for qc in range(QCH):
                u_next = sc_exp(qc + 1) if qc + 1 < QCH else None
                rrec = softmax_tail(qc, u_cur)
                av(qc, u_cur, rrec)
                if tail:
                    oproj_group(qc)
                u_cur = u_next

        # ---- phase 1 (+ interleaved attention) ---------------------------
        with (
            tc.tile_pool(name="p1x", bufs=1) as p1x,
            tc.tile_pool(name="p1wv", bufs=1) as p1wv,
            tc.tile_pool(name="p1cs", bufs=1) as p1cs,
            tc.tile_pool(name="p1w", bufs=1) as p1w,
            tc.tile_pool(name="p1rot", bufs=1) as p1rot,
            tc.tile_pool(name="p1stg", bufs=1) as p1stg,
        ):
            # Loads issued in first-use order: the first V chain needs wv
            # chunk 0 + x tile 0 only (~2.6MB), so the PE starts ~10us in.
            wv0 = p1wv.tile([128, CT, 512], f16, tag="wvh", name="wvh0")
            nc.sync.dma_start(out=wv0[:], in_=wv_pack.ap()[:, 0])
            x_sb = p1x.tile([128, TT, CT, 128], f16, tag="xt")
            for tm in range(TT):
                nc.sync.dma_start(
                    out=x_sb[:, tm], in_=x_pack.ap()[:, tm]
                )

            def load_w(h):
                w_h = {}
                for nm, pack in (("q", wq_pack), ("k", wk_pack)):
                    w = p1w.tile(
                        [128, CT, 128], f16, tag=f"w{nm}", bufs=2,
                        name=f"w{nm}{h}"
                    )
                    nc.sync.dma_start(out=w[:], in_=pack.ap()[h])
                    w_h[nm] = w
                return w_h

            w_next = load_w(0)
            cs_sb = p1cs.tile([128, T], f16, tag="cs")
            nc.sync.dma_start(out=cs_sb[:], in_=cs_pack.ap())

            def v_block(dch, wv_h):
                for tm in range(TT):
                    ps = ps_gen.tile(
                        [128, 512], f32, tag="ps", name=f"vps{dch}_{tm}"
                    )
                    for ci in range(CT):
                        nc.tensor.matmul(
                            ps[:],
                            lhsT=x_sb[:, tm, ci, :],
                            rhs=wv_h[:, ci, :],
                            start=(ci == 0),
                            stop=(ci == CT - 1),
                        )
                    vstg = p1stg.tile(
                        [128, 512], f16, tag="vstg", bufs=4,
                        name=f"vstg{dch}_{tm}"
                    )
                    nc.scalar.copy(out=vstg[:], in_=ps[:])
                    for hh in range(4):
                        nc.sync.dma_start(
                            out=v_dram[4 * dch + hh][:, tm, :],
                            in_=vstg[:, ts(hh, 128)],
                        )

            v_block(0, wv0)

            def qk_block(h, w_h):
                qt = p2q.tile([128, T], f16, tag="qt", name=f"qt{h}")
                kt = p2k.tile([128, T], f16, tag="kt", name=f"kt{h}")
                for nm, dst in (("q", qt), ("k", kt)):
                    for tch in range(QCH):
                        ps = ps_gen.tile(
                            [128, 512], f32, tag="ps", name=f"qk{h}{nm}{tch}"
                        )
                        for ci in range(CT):
                            nc.tensor.matmul(
                                ps[:],
                                lhsT=w_h[nm][:, ci, :],
                                rhs=x_sb[:, ts(tch, 4), ci, :],
                                start=(ci == 0),
                                stop=(ci == CT - 1),
                            )
                        # out1 = x1*cos + x2*sin ; out2 = x1*cos - x2*sin
                        t12 = p1rot.tile([64, 2, 512], f32, tag="t12")
                        t1 = t12[:, 0, :]
                        t2 = t12[:, 1, :]
                        nc.vector.tensor_mul(
                            t1, ps[0:64, :], cs_sb[0:64, ts(tch, 512)]
                        )
                        nc.vector.tensor_mul(
                            t2, ps[64:128, :], cs_sb[64:128, ts(tch, 512)]
                        )
                        nc.vector.tensor_add(dst[0:64, ts(tch, 512)], t1, t2)
                        nc.vector.tensor_sub(dst[64:128, ts(tch, 512)], t1, t2)
                return qt, kt

            qk_tiles = {}
            for h in range(HG):
                w_h = w_next
                qk_tiles[h] = qk_block(h, w_h)
                if h + 1 < HG:
                    w_next = load_w(h + 1)
                if h == 0:
                    wv1 = p1wv.tile(
                        [128, CT, 512], f16, tag="wvh", name="wvh1"
                    )
                    nc.sync.dma_start(out=wv1[:], in_=wv_pack.ap()[:, 1])
                    v_block(1, wv1)
                if h >= 1:
                    qt, kt = qk_tiles.pop(h - 1)
                    attention(h - 1, qt, kt)

        # o-projection pools: opened after phase 1 releases its SBUF (they
        # alias that zone; wo is loaded per-head so the ji=0 slice lands
        # within ~2us of the release and o-proj never waits on the 4MB load)
        p3wo = ctx.enter_context(tc.tile_pool(name="p3wo", bufs=1))
        p3stg = ctx.enter_context(tc.tile_pool(name="p3stg", bufs=1))
        wo_sb = p3wo.tile([128, HG, C], f16)
        for ji in range(HG):
            nc.sync.dma_start(out=wo_sb[:, ji, :], in_=wo_pack.ap()[:, ji, :])

        # last head's attention (+ interleaved o-projection groups)
        qt, kt = qk_tiles.pop(HG - 1)
        attention(HG - 1, qt, kt, tail=True)

    nc.compile()
    return nc


def get_nc():
    if "nc" not in _CACHE:
        _CACHE["nc"] = _build_bass()
    return _CACHE["nc"]


def _pack_inputs(x, cos, sin, wq, wk, wv, wo):
    """Build the 8 per-core input maps (packed, DMA-friendly fp16 layouts)."""
    cs = np.concatenate(
        [
            np.asarray(cos[0, :, 0, :], dtype=np.float32).T,  # (64, T)
            np.asarray(sin[0, :, 0, :], dtype=np.float32).T,
        ],
        axis=0,
    )  # (128, T)
    cs = np.ascontiguousarray(cs.astype(F16))
    in_maps = []
    for core in range(N_CORES):
        b, g = divmod(core, 2)
        xb = np.asarray(x[b], dtype=np.float32)  # (T, C)
        # x_pack[ci, tm, co, tq] = x[b, tm*128+tq, co*128+ci]
        x_pack = np.ascontiguousarray(
            xb.reshape(TT, 128, CT, 128).transpose(3, 0, 2, 1).astype(F16)
        )
        sl = slice(g * JG, (g + 1) * JG)
        wq_g = np.asarray(wq[sl], dtype=np.float32)  # (JG, C)
        wk_g = np.asarray(wk[sl], dtype=np.float32)
        wv_g = np.asarray(wv[sl], dtype=np.float32)
        wo_g = np.asarray(wo[:, sl], dtype=np.float32)  # (C, JG)
        # wq_pack[h, ci, co, d] = wq_g[h*128+d, co*128+ci]
        wq_pack = np.ascontiguousarray(
            wq_g.reshape(HG, 128, CT, 128).transpose(0, 3, 2, 1).astype(F16)
        )
        wk_pack = np.ascontiguousarray(
            wk_g.reshape(HG, 128, CT, 128).transpose(0, 3, 2, 1).astype(F16)
        )
        # wv_pack[ci, dch, co, dq] = wv_g[dch*512+dq, co*128+ci]
        wv_pack = np.ascontiguousarray(
            wv_g.reshape(2, 512, CT, 128).transpose(3, 0, 2, 1).astype(F16)
        )
        # wo_pack[ji, jo, c] = wo_g[c, jo*128+ji]
        wo_pack = np.ascontiguousarray(
            wo_g.reshape(C, HG, 128).transpose(2, 1, 0).astype(F16)
        )
        in_maps.append(
            {
                "x_pack": x_pack,
                "wq_pack": wq_pack,
                "wk_pack": wk_pack,
                "wv_pack": wv_pack,
                "wo_pack": wo_pack,
                "cs_pack": cs,
            }
        )
    return in_maps


def run_spmd(in_maps, **kwargs):
    from concourse.bass_utils import run_bass_kernel_spmd

    nc = get_nc()
    return run_bass_kernel_spmd(nc, in_maps, core_ids=list(range(N_CORES)), **kwargs)


def kernel(x, cos, sin, wq, wk, wv, wo):
    in_maps = _pack_inputs(x, cos, sin, wq, wk, wv, wo)
    res = run_spmd(in_maps)
    outs = [r["out"] for r in res.results]
    full = np.empty((B, T, C), dtype=np.float32)
    for b in range(B):
        full[b] = outs[2 * b] + outs[2 * b + 1]
    return full
